# revision 22
# baseline (speedup 1.0000x reference)
"""Trainium2 Bass kernel for a top-2 ternary-weight MoE FFN.

Sharding: expert-parallel over 8 NeuronCores (1 expert/core). A first
SPMD program computes fp16 router logits on-device (each core routes its
own 1/8 token slice) and returns the full logit rows; the host does
top-2 + sigmoid combine weights, repairing near-tie tokens (rank-2/3
logit gap below threshold) with an exact fp32 recompute so routing
matches the fp32 reference. The host then performs the all-to-all,
routing each token's row to the core(s) owning its selected experts.

The second SPMD program runs the expert FFN entirely in fp16: the host
ternarizes the fp32 weights (threshold = per-matrix median of |w|,
values {-1,0,+1} are exact in fp16) and uploads them pre-transposed in
the PE lhsT layout, so the device is a pure matmul pipeline:
gate -> silu -> up -> m -> down, software-pipelined so the PE never
stalls (down-projection of tile i runs between the gate and up phases
of tile i+1). w_down is pre-scaled by 1/4 (exact) so the fp16 outputs
cannot overflow; the host folds the 4x back into the combine weights
while unsharding (summing the two expert contributions per token).
"""

import os

import numpy as np

import concourse.bacc as bacc
import concourse.mybir as mybir
from concourse.tile import TileContext
from concourse.bass_utils import run_bass_kernel_spmd

FP32 = mybir.dt.float32
F16 = mybir.dt.float16

NCORES = 8
B, T, D, H, E = 4, 2048, 1024, 2048, 8
N = B * T                    # 8192 tokens
TSLICE = N // NCORES         # tokens routed per core in phase A
KO_D = D // 128              # 8 contraction chunks over D
KO_H = H // 128              # 16 contraction chunks over H
HC = H // 128                # 16 output chunks over H (gate/up)
DC = D // 128                # 8 output chunks over D (down)

LAST_HW_NS = None
LAST_PHASE_NS = None

_program_cache = {}


def _ensure_ntff_hook():
    """Profiling-only: register the axon NTFF hook that the trimmed antenv
    package lacks, and stub out artifact upload (no bucket creds here)."""
    import sys
    import types

    import concourse.bass_utils as bu
    bu.upload_artifacts = lambda d: str(d)
    try:
        from antenv.axon_hooks import get_axon_ntff_profile_hook
        if get_axon_ntff_profile_hook() is not None:
            return
    except ImportError:
        mod = types.ModuleType("antenv.axon_hooks")
        box = {}
        mod.set_axon_ntff_profile_hook = lambda h: box.__setitem__("h", h)
        mod.get_axon_ntff_profile_hook = lambda: box.get("h")
        sys.modules["antenv.axon_hooks"] = mod
        import antenv
        antenv.axon_hooks = mod
    from antenv.axon_hooks import set_axon_ntff_profile_hook
    from trn_agent_boot.trn_boot import _ntff_profile_via_ctypes
    set_axon_ntff_profile_hook(
        _ntff_profile_via_ctypes("/opt/axon/libaxon_pjrt.so"))


def _run(nc, in_maps, label):
    trace = bool(int(os.environ.get("MOE_TRACE", "0")))
    kw = {}
    if trace:
        _ensure_ntff_hook()
        kw = dict(trace=True, trace_cores=list(range(NCORES)),
                  trace_kwargs={"title": label})
    res = run_bass_kernel_spmd(nc, in_maps, core_ids=list(range(NCORES)), **kw)
    if trace:
        global LAST_PHASE_NS
        print(f"[{label}] exec_time_ns={res.exec_time_ns} "
              f"mean={res.mean_exec_time_ns} "
              f"slowest_core={res.max_exec_time_core_id} "
              f"trace={res.instructions_and_trace[1] if res.instructions_and_trace else None}")
        if res.exec_time_ns:
            LAST_PHASE_NS[label] = res.exec_time_ns
    return res


def _build_router():
    """Phase A: fp16 logits for this core's token slice, returned as full
    [E, TSLICE] fp32 rows (host does top-2; near-ties repaired exactly)."""
    nc = bacc.Bacc("TRN2", target_bir_lowering=False, debug=False,
                   num_devices=NCORES)
    xt = nc.dram_tensor("xt", [128, KO_D, TSLICE], F16, kind="ExternalInput")
    rwt = nc.dram_tensor("rwt", [128, KO_D, E], F16, kind="ExternalInput")
    lg = nc.dram_tensor("lg", [E, TSLICE], FP32, kind="ExternalOutput")

    with TileContext(nc) as tc:
        with (
            tc.tile_pool(name="sbuf", bufs=2) as pool,
            tc.tile_pool(name="cpool", bufs=1) as cpool,
            tc.tile_pool(name="ps", bufs=2, space="PSUM") as ps,
        ):
            rwt_sb = cpool.tile([128, KO_D, E], F16)
            nc.sync.dma_start(rwt_sb[:], rwt.ap()[:, :, :])
            # one SBUF tile per k-chunk: tile-granular dependency tracking
            # means matmul k only waits for its own chunk's DMA; alternate
            # queues since each dma_start costs ~0.6us of issue time
            xks = []
            for k in range(KO_D):
                xk = cpool.tile([128, TSLICE], F16, tag=f"xk{k}",
                                name=f"xk{k}")
                eng = nc.sync if k % 2 == 0 else nc.gpsimd
                eng.dma_start(xk[:], xt.ap()[:, k, :])
                xks.append(xk)
            for t in range(TSLICE // 512):
                sl = slice(t * 512, (t + 1) * 512)
                pl = ps.tile([E, 512], FP32, tag="pl")
                for k in range(KO_D):
                    nc.tensor.matmul(pl[:], lhsT=rwt_sb[:, k, :],
                                     rhs=xks[k][:, sl],
                                     start=(k == 0), stop=(k == KO_D - 1))
                ls = pool.tile([E, 512], FP32, tag="ls")
                nc.vector.tensor_copy(ls[:], pl[:])
                nc.sync.dma_start(lg.ap()[:, sl], ls[:])
    nc.compile()
    return nc


def _token_tiles(cap):
    tiles = []
    t0 = 0
    while t0 < cap:
        tsz = min(512, cap - t0)
        tiles.append((t0, tsz))
        t0 += tsz
    return tiles


def _build_ffn(cap):
    """Phase B: per-core expert FFN over `cap` gathered token rows.

    inputs (all fp16, host-prepared):
      wg/wu [128, HC, KO_D*128]  ternary gate/up in lhsT layout
      wd    [128, DC, KO_H*128]  ternary down (x 1/4) in lhsT layout
      xg    [128, KO_D, cap]     token rows in rhs layout
    output:
      yt [D, cap] fp16: unweighted expert outputs (x 1/4), transposed.

    Pipeline per 512-token tile: gate matmuls -> silu (ACT) into sg;
    down-projection of the PREVIOUS tile (its m is long since ready, so
    the PE never waits); up matmuls -> m = sg*pu (DVE) in fp16.
    """
    nc = bacc.Bacc("TRN2", target_bir_lowering=False, debug=False,
                   num_devices=NCORES)
    wg = nc.dram_tensor("wg", [128, HC, KO_D * 128], F16,
                        kind="ExternalInput")
    wu = nc.dram_tensor("wu", [128, HC, KO_D * 128], F16,
                        kind="ExternalInput")
    wd = nc.dram_tensor("wd", [128, DC, KO_H * 128], F16,
                        kind="ExternalInput")
    # tile-major token/output layouts: one DMA issue per tile with 8 KB
    # contiguous per-partition lines (1 KB lines cost ~5x in transfer time)
    nt = len(_token_tiles(cap))
    xg = nc.dram_tensor("xg", [nt, 128, KO_D, 512], F16,
                        kind="ExternalInput")
    yt = nc.dram_tensor("yt", [nt, 128, DC, 512], F16,
                        kind="ExternalOutput")

    with TileContext(nc) as tc:
        with (
            tc.tile_pool(name="wpool", bufs=1) as wpool,
            tc.tile_pool(name="xpool", bufs=2) as xpool,
            tc.tile_pool(name="spool", bufs=1) as spool,
            tc.tile_pool(name="mpool", bufs=2) as mpool,
            tc.tile_pool(name="ypool", bufs=2) as ypool,
            tc.tile_pool(name="psg", bufs=2, space="PSUM") as psg,
            tc.tile_pool(name="psu", bufs=2, space="PSUM") as psu,
            tc.tile_pool(name="pso", bufs=2, space="PSUM") as pso,
        ):
            # ternary fp16 weights, SBUF-resident for the whole kernel.
            # Chunked DMAs (256 KB each) so the first gate matmul only
            # waits on the first chunk; weights ride the SWDGE queue so
            # they don't delay token loads / output stores on sync HWDGE.
            # one SBUF tile per weight chunk: tile-granular dependency
            # tracking means the first gate matmul only waits on chunk 0's
            # DMA instead of all 16
            wg_sb, wu_sb, wd_sb = [], [], []
            for h in range(HC):
                t = wpool.tile([128, KO_D * 128], F16, name=f"wg{h}")
                nc.gpsimd.dma_start(t[:], wg.ap()[:, h, :])
                wg_sb.append(t)
            for h in range(HC):
                t = wpool.tile([128, KO_D * 128], F16, name=f"wu{h}")
                nc.gpsimd.dma_start(t[:], wu.ap()[:, h, :])
                wu_sb.append(t)
            for d in range(DC):
                t = wpool.tile([128, KO_H * 128], F16, name=f"wd{d}")
                nc.gpsimd.dma_start(t[:], wd.ap()[:, d, :])
                wd_sb.append(t)

            sg_sb = spool.tile([128, HC, 512], F16)

            def emit_down(m_t, ti, tsz):
                # all 8 d-chunks collected in one SBUF tile -> ONE output
                # DMA issue per tile (each dma_start costs ~0.6us of
                # serialized issue time on the sync queue); columns beyond
                # tsz carry junk the host ignores
                ysb = ypool.tile([128, DC, 512], F16, tag="y")
                for d in range(DC):
                    po = pso.tile([128, 512], FP32, tag="po")
                    for k in range(KO_H):
                        nc.tensor.matmul(po[:, :tsz],
                                         lhsT=wd_sb[d][:,
                                                    k * 128:(k + 1) * 128],
                                         rhs=m_t[:, k, :tsz],
                                         start=(k == 0),
                                         stop=(k == KO_H - 1))
                    nc.vector.tensor_copy(ysb[:, d, :tsz], po[:, :tsz])
                nc.sync.dma_start(yt.ap()[ti, :, :, :], ysb[:])

            tiles = _token_tiles(cap)
            xts = {}
            prev = None
            for ti, (t0, tsz) in enumerate(tiles):
                if ti == 0:
                    xts[0] = xpool.tile([128, KO_D, 512], F16, tag="x", name="xt_sb")
                    nc.sync.dma_start(xts[0][:], xg.ap()[0, :, :, :])
                xt_sb = xts.pop(ti)
                # phase 1: gate -> silu
                for h in range(HC):
                    pg = psg.tile([128, 512], FP32, tag="pg")
                    for k in range(KO_D):
                        nc.tensor.matmul(pg[:, :tsz],
                                         lhsT=wg_sb[h][:,
                                                    k * 128:(k + 1) * 128],
                                         rhs=xt_sb[:, k, :tsz],
                                         start=(k == 0),
                                         stop=(k == KO_D - 1))
                    nc.scalar.activation(sg_sb[:, h, :tsz], pg[:, :tsz],
                                         mybir.ActivationFunctionType.Silu)
                # prefetch next tile's tokens during this tile's back half
                if ti + 1 < len(tiles):
                    xts[ti + 1] = xpool.tile([128, KO_D, 512], F16, tag="x",
                                             name="xt_sb")
                    nc.sync.dma_start(xts[ti + 1][:], xg.ap()[ti + 1, :, :, :])
                # down-projection of the previous tile (m ready long ago)
                if prev is not None:
                    emit_down(*prev)
                # phase 2: up -> m = sg * pu
                m_t = mpool.tile([128, KO_H, 512], F16, tag="m")
                for h in range(HC):
                    pu = psu.tile([128, 512], FP32, tag="pu")
                    for k in range(KO_D):
                        nc.tensor.matmul(pu[:, :tsz],
                                         lhsT=wu_sb[h][:,
                                                    k * 128:(k + 1) * 128],
                                         rhs=xt_sb[:, k, :tsz],
                                         start=(k == 0),
                                         stop=(k == KO_D - 1))
                    nc.vector.tensor_tensor(out=m_t[:, h, :tsz],
                                            in0=sg_sb[:, h, :tsz],
                                            in1=pu[:, :tsz],
                                            op=mybir.AluOpType.mult)
                prev = (m_t, ti, tsz)
            emit_down(*prev)
    nc.compile()
    return nc


def _get_program(key):
    if key not in _program_cache:
        _program_cache[key] = _build_router() if key == "router" \
            else _build_ffn(key)
    return _program_cache[key]


def _lhsT_layout(wt, ko, oc):
    """[K, M] fp16 -> [128, M/128, K/128*128] lhsT chunk layout."""
    return np.ascontiguousarray(
        wt.reshape(ko, 128, oc, 128).transpose(1, 2, 0, 3)
        .reshape(128, oc, ko * 128))


def _ternary16(w):
    a = np.float32(np.median(np.abs(w)))
    return ((w > a).astype(np.float16) - (w < -a).astype(np.float16))


def kernel(x, router_w, w_gate, w_up, w_down, top_k):
    assert int(top_k) == 2
    xf = np.ascontiguousarray(x.reshape(N, D).astype(np.float32))
    xf16 = xf.astype(np.float16)

    # ---- phase A: on-device fp16 logits; host top-2 + exact tie repair ----
    global LAST_HW_NS, LAST_PHASE_NS
    LAST_PHASE_NS = {}
    rnc = _get_program("router")
    rwt16 = router_w.T.astype(np.float16)                      # [D, E]
    rwt_r = np.ascontiguousarray(
        rwt16.reshape(KO_D, 128, E).transpose(1, 0, 2))
    in_maps = [
        {"xt": np.ascontiguousarray(
            xf16[c * TSLICE:(c + 1) * TSLICE].T
            .reshape(KO_D, 128, TSLICE).transpose(1, 0, 2)),
         "rwt": rwt_r}
        for c in range(NCORES)
    ]
    rres = _run(rnc, in_maps, "router")
    L = np.concatenate([r["lg"].T for r in rres.results],
                       axis=0).astype(np.float32)              # [N, E]
    order = np.argsort(-L, axis=1, kind="stable")
    l2 = np.take_along_axis(L, order[:, 1:2], 1)[:, 0]
    l3 = np.take_along_axis(L, order[:, 2:3], 1)[:, 0]
    # fp16-logit error is ~4e-4; repair any token whose expert SET could
    # differ from the fp32 reference's top-2 with exact logits.
    bad = np.nonzero(l2 - l3 < 4e-3)[0]
    if bad.size:
        Lx = xf[bad] @ router_w.astype(np.float32).T
        L[bad] = Lx
        order[bad] = np.argsort(-Lx, axis=1, kind="stable")
    e1 = order[:, 0]
    e2 = order[:, 1]
    ar = np.arange(N)
    w1 = (1.0 / (1.0 + np.exp(-(L[ar, e1] - L[ar, e2])))).astype(np.float32)
    w2 = np.float32(1.0) - w1

    # ---- host all-to-all: token rows -> expert cores ----
    toks, wts = [], []
    for e in range(E):
        sel = np.nonzero((e1 == e) | (e2 == e))[0]
        toks.append(sel)
        wts.append(np.where(e1[sel] == e, w1[sel], w2[sel]).astype(np.float32))
    counts = [len(s) for s in toks]
    # every core runs `cap` rows (exec time = slowest core), so use the
    # exact max count instead of rounding up to a multiple of 128
    cap = max(max(counts), 512)

    fnc = _get_program(cap)
    nt = len(_token_tiles(cap))
    in_maps = []
    for e in range(E):
        xgp = np.zeros((nt * 512, D), dtype=np.float16)
        xgp[:counts[e]] = xf16[toks[e]]
        # tile-major [nt, 128, KO_D, 512] so each tile is one contiguous DMA
        xg_t = np.ascontiguousarray(
            xgp.T.reshape(KO_D, 128, nt, 512).transpose(2, 1, 0, 3))
        wgq = _ternary16(np.asarray(w_gate[e], dtype=np.float32))  # [H, D]
        wuq = _ternary16(np.asarray(w_up[e], dtype=np.float32))    # [H, D]
        wdq = _ternary16(np.asarray(w_down[e], dtype=np.float32))  # [D, H]
        wdq *= np.float16(0.25)   # exact; keeps fp16 outputs in range
        in_maps.append({
            "wg": _lhsT_layout(wgq.T, KO_D, HC),
            "wu": _lhsT_layout(wuq.T, KO_D, HC),
            "wd": _lhsT_layout(wdq.T, KO_H, DC),
            "xg": xg_t,
        })
    fres = _run(fnc, in_maps, "ffn")
    if LAST_PHASE_NS:
        LAST_HW_NS = sum(LAST_PHASE_NS.values())

    # ---- unshard: weighted sum of the (<= 2) expert contributions ----
    out = np.zeros((N, D), dtype=np.float32)
    for e in range(E):
        yt_t = fres.results[e]["yt"]                  # [nt, 128, DC, 512]
        y = yt_t.transpose(2, 1, 0, 3).reshape(D, nt * 512)
        ytc = y[:, :counts[e]].T.astype(np.float32)
        out[toks[e]] += (4.0 * wts[e])[:, None] * ytc
    return out.reshape(B, T, D)


# revision 25
# speedup vs baseline: 1.0036x; 1.0036x over previous
"""Trainium2 Bass kernel for a top-2 ternary-weight MoE FFN.

Sharding: expert-parallel over 8 NeuronCores (1 expert/core). A first
SPMD program computes fp16 router logits on-device (each core routes its
own 1/8 token slice) and returns the full logit rows; the host does
top-2 + sigmoid combine weights, repairing near-tie tokens (rank-2/3
logit gap below threshold) with an exact fp32 recompute so routing
matches the fp32 reference. The host then performs the all-to-all,
routing each token's row to the core(s) owning its selected experts.

The second SPMD program runs the expert FFN entirely in fp16: the host
ternarizes the fp32 weights (threshold = per-matrix median of |w|,
values {-1,0,+1} are exact in fp16) and uploads them pre-transposed in
the PE lhsT layout, so the device is a pure matmul pipeline:
gate -> silu -> up -> m -> down, software-pipelined so the PE never
stalls (down-projection of tile i runs between the gate and up phases
of tile i+1). w_down is pre-scaled by 1/4 (exact) so the fp16 outputs
cannot overflow; the host folds the 4x back into the combine weights
while unsharding (summing the two expert contributions per token).
"""

import os

import numpy as np

import concourse.bacc as bacc
import concourse.mybir as mybir
from concourse.tile import TileContext
from concourse.bass_utils import run_bass_kernel_spmd

FP32 = mybir.dt.float32
F16 = mybir.dt.float16

NCORES = 8
B, T, D, H, E = 4, 2048, 1024, 2048, 8
N = B * T                    # 8192 tokens
TSLICE = N // NCORES         # tokens routed per core in phase A
KO_D = D // 128              # 8 contraction chunks over D
KO_H = H // 128              # 16 contraction chunks over H
HC = H // 128                # 16 output chunks over H (gate/up)
DC = D // 128                # 8 output chunks over D (down)

LAST_HW_NS = None
LAST_PHASE_NS = None

_program_cache = {}


def _ensure_ntff_hook():
    """Profiling-only: register the axon NTFF hook that the trimmed antenv
    package lacks, and stub out artifact upload (no bucket creds here)."""
    import sys
    import types

    import concourse.bass_utils as bu
    bu.upload_artifacts = lambda d: str(d)
    try:
        from antenv.axon_hooks import get_axon_ntff_profile_hook
        if get_axon_ntff_profile_hook() is not None:
            return
    except ImportError:
        mod = types.ModuleType("antenv.axon_hooks")
        box = {}
        mod.set_axon_ntff_profile_hook = lambda h: box.__setitem__("h", h)
        mod.get_axon_ntff_profile_hook = lambda: box.get("h")
        sys.modules["antenv.axon_hooks"] = mod
        import antenv
        antenv.axon_hooks = mod
    from antenv.axon_hooks import set_axon_ntff_profile_hook
    from trn_agent_boot.trn_boot import _ntff_profile_via_ctypes
    set_axon_ntff_profile_hook(
        _ntff_profile_via_ctypes("/opt/axon/libaxon_pjrt.so"))


def _run(nc, in_maps, label):
    trace = bool(int(os.environ.get("MOE_TRACE", "0")))
    kw = {}
    if trace:
        _ensure_ntff_hook()
        kw = dict(trace=True, trace_cores=list(range(NCORES)),
                  trace_kwargs={"title": label})
    res = run_bass_kernel_spmd(nc, in_maps, core_ids=list(range(NCORES)), **kw)
    if trace:
        global LAST_PHASE_NS
        print(f"[{label}] exec_time_ns={res.exec_time_ns} "
              f"mean={res.mean_exec_time_ns} "
              f"slowest_core={res.max_exec_time_core_id} "
              f"trace={res.instructions_and_trace[1] if res.instructions_and_trace else None}")
        if res.exec_time_ns:
            LAST_PHASE_NS[label] = res.exec_time_ns
    return res


def _build_router():
    """Phase A: fp16 logits for this core's token slice, returned as full
    [E, TSLICE] fp32 rows (host does top-2; near-ties repaired exactly)."""
    nc = bacc.Bacc("TRN2", target_bir_lowering=False, debug=False,
                   num_devices=NCORES)
    xt = nc.dram_tensor("xt", [128, KO_D, TSLICE], F16, kind="ExternalInput")
    rwt = nc.dram_tensor("rwt", [128, KO_D, E], F16, kind="ExternalInput")
    lg = nc.dram_tensor("lg", [E, TSLICE], FP32, kind="ExternalOutput")

    with TileContext(nc) as tc:
        with (
            tc.tile_pool(name="sbuf", bufs=2) as pool,
            tc.tile_pool(name="cpool", bufs=1) as cpool,
            tc.tile_pool(name="ps", bufs=2, space="PSUM") as ps,
        ):
            rwt_sb = cpool.tile([128, KO_D, E], F16)
            nc.sync.dma_start(rwt_sb[:], rwt.ap()[:, :, :])
            # one SBUF tile per pair of k-chunks: tile-granular dependency
            # tracking means matmul k only waits for its own chunk's DMA;
            # all on the HWDGE queue (SWDGE completion semaphores are slow)
            xks = []
            for j in range(KO_D // 2):
                xk = cpool.tile([128, 2, TSLICE], F16, tag=f"xk{j}",
                                name=f"xk{j}")
                nc.sync.dma_start(xk[:], xt.ap()[:, 2 * j:2 * j + 2, :])
                xks.append(xk)
            for t in range(TSLICE // 512):
                sl = slice(t * 512, (t + 1) * 512)
                pl = ps.tile([E, 512], FP32, tag="pl")
                for k in range(KO_D):
                    nc.tensor.matmul(pl[:], lhsT=rwt_sb[:, k, :],
                                     rhs=xks[k // 2][:, k % 2, sl],
                                     start=(k == 0), stop=(k == KO_D - 1))
                ls = pool.tile([E, 512], FP32, tag="ls")
                nc.vector.tensor_copy(ls[:], pl[:])
                nc.sync.dma_start(lg.ap()[:, sl], ls[:])
    nc.compile()
    return nc


def _token_tiles(cap):
    tiles = []
    t0 = 0
    while t0 < cap:
        tsz = min(512, cap - t0)
        tiles.append((t0, tsz))
        t0 += tsz
    return tiles


def _build_ffn(cap):
    """Phase B: per-core expert FFN over `cap` gathered token rows.

    inputs (all fp16, host-prepared):
      wg/wu [128, HC, KO_D*128]  ternary gate/up in lhsT layout
      wd    [128, DC, KO_H*128]  ternary down (x 1/4) in lhsT layout
      xg    [128, KO_D, cap]     token rows in rhs layout
    output:
      yt [D, cap] fp16: unweighted expert outputs (x 1/4), transposed.

    Pipeline per 512-token tile: gate matmuls -> silu (ACT) into sg;
    down-projection of the PREVIOUS tile (its m is long since ready, so
    the PE never waits); up matmuls -> m = sg*pu (DVE) in fp16.
    """
    nc = bacc.Bacc("TRN2", target_bir_lowering=False, debug=False,
                   num_devices=NCORES)
    wg = nc.dram_tensor("wg", [128, HC, KO_D * 128], F16,
                        kind="ExternalInput")
    wu = nc.dram_tensor("wu", [128, HC, KO_D * 128], F16,
                        kind="ExternalInput")
    wd = nc.dram_tensor("wd", [128, DC, KO_H * 128], F16,
                        kind="ExternalInput")
    # tile-major token/output layouts: one DMA issue per tile with 8 KB
    # contiguous per-partition lines (1 KB lines cost ~5x in transfer time)
    nt = len(_token_tiles(cap))
    xg = nc.dram_tensor("xg", [nt, 128, KO_D, 512], F16,
                        kind="ExternalInput")
    yt = nc.dram_tensor("yt", [nt, 128, DC, 512], F16,
                        kind="ExternalOutput")

    with TileContext(nc) as tc:
        with (
            tc.tile_pool(name="wpool", bufs=1) as wpool,
            tc.tile_pool(name="xpool", bufs=2) as xpool,
            tc.tile_pool(name="spool", bufs=1) as spool,
            tc.tile_pool(name="mpool", bufs=2) as mpool,
            tc.tile_pool(name="ypool", bufs=2) as ypool,
            tc.tile_pool(name="psg", bufs=2, space="PSUM") as psg,
            tc.tile_pool(name="psu", bufs=2, space="PSUM") as psu,
            tc.tile_pool(name="pso", bufs=2, space="PSUM") as pso,
        ):
            # ternary fp16 weights, SBUF-resident for the whole kernel.
            # Chunked DMAs (256 KB each) so the first gate matmul only
            # waits on the first chunk; weights ride the SWDGE queue so
            # they don't delay token loads / output stores on sync HWDGE.
            # one SBUF tile per weight chunk: tile-granular dependency
            # tracking means the first gate matmul only waits on chunk 0's
            # DMA instead of all 16. The first chunks ride the sync HWDGE
            # queue (SWDGE completion semaphores arrive several us late).
            xts = {}
            xts[0] = xpool.tile([128, KO_D, 512], F16, tag="x",
                                name="xt_sb")
            nc.sync.dma_start(xts[0][:], xg.ap()[0, :, :, :])
            wg_sb, wu_sb, wd_sb = [], [], []
            for h in range(HC):
                t = wpool.tile([128, KO_D * 128], F16, name=f"wg{h}")
                eng = nc.sync if h < 4 else nc.gpsimd
                eng.dma_start(t[:], wg.ap()[:, h, :])
                wg_sb.append(t)
            for h in range(HC):
                t = wpool.tile([128, KO_D * 128], F16, name=f"wu{h}")
                nc.gpsimd.dma_start(t[:], wu.ap()[:, h, :])
                wu_sb.append(t)
            for d in range(DC):
                t = wpool.tile([128, KO_H * 128], F16, name=f"wd{d}")
                nc.gpsimd.dma_start(t[:], wd.ap()[:, d, :])
                wd_sb.append(t)

            sg_sb = spool.tile([128, HC, 512], F16)

            def emit_down(m_t, ti, tsz):
                # all 8 d-chunks collected in one SBUF tile -> ONE output
                # DMA issue per tile (each dma_start costs ~0.6us of
                # serialized issue time on the sync queue); columns beyond
                # tsz carry junk the host ignores
                ysb = ypool.tile([128, DC, 512], F16, tag="y")
                for d in range(DC):
                    po = pso.tile([128, 512], FP32, tag="po")
                    for k in range(KO_H):
                        nc.tensor.matmul(po[:, :tsz],
                                         lhsT=wd_sb[d][:,
                                                    k * 128:(k + 1) * 128],
                                         rhs=m_t[:, k, :tsz],
                                         start=(k == 0),
                                         stop=(k == KO_H - 1))
                    nc.vector.tensor_copy(ysb[:, d, :tsz], po[:, :tsz])
                nc.sync.dma_start(yt.ap()[ti, :, :, :], ysb[:])

            tiles = _token_tiles(cap)
            prev = None
            for ti, (t0, tsz) in enumerate(tiles):
                xt_sb = xts.pop(ti)
                # phase 1: gate -> silu
                for h in range(HC):
                    pg = psg.tile([128, 512], FP32, tag="pg")
                    for k in range(KO_D):
                        nc.tensor.matmul(pg[:, :tsz],
                                         lhsT=wg_sb[h][:,
                                                    k * 128:(k + 1) * 128],
                                         rhs=xt_sb[:, k, :tsz],
                                         start=(k == 0),
                                         stop=(k == KO_D - 1))
                    nc.scalar.activation(sg_sb[:, h, :tsz], pg[:, :tsz],
                                         mybir.ActivationFunctionType.Silu)
                # prefetch next tile's tokens during this tile's back half
                if ti + 1 < len(tiles):
                    xts[ti + 1] = xpool.tile([128, KO_D, 512], F16, tag="x",
                                             name="xt_sb")
                    nc.sync.dma_start(xts[ti + 1][:], xg.ap()[ti + 1, :, :, :])
                # down-projection of the previous tile (m ready long ago)
                if prev is not None:
                    emit_down(*prev)
                # phase 2: up -> m = sg * pu
                m_t = mpool.tile([128, KO_H, 512], F16, tag="m")
                for h in range(HC):
                    pu = psu.tile([128, 512], FP32, tag="pu")
                    for k in range(KO_D):
                        nc.tensor.matmul(pu[:, :tsz],
                                         lhsT=wu_sb[h][:,
                                                    k * 128:(k + 1) * 128],
                                         rhs=xt_sb[:, k, :tsz],
                                         start=(k == 0),
                                         stop=(k == KO_D - 1))
                    nc.vector.tensor_tensor(out=m_t[:, h, :tsz],
                                            in0=sg_sb[:, h, :tsz],
                                            in1=pu[:, :tsz],
                                            op=mybir.AluOpType.mult)
                prev = (m_t, ti, tsz)
            emit_down(*prev)
    nc.compile()
    return nc


def _get_program(key):
    if key not in _program_cache:
        _program_cache[key] = _build_router() if key == "router" \
            else _build_ffn(key)
    return _program_cache[key]


def _lhsT_layout(wt, ko, oc):
    """[K, M] fp16 -> [128, M/128, K/128*128] lhsT chunk layout."""
    return np.ascontiguousarray(
        wt.reshape(ko, 128, oc, 128).transpose(1, 2, 0, 3)
        .reshape(128, oc, ko * 128))


def _ternary16(w):
    a = np.float32(np.median(np.abs(w)))
    return ((w > a).astype(np.float16) - (w < -a).astype(np.float16))


def kernel(x, router_w, w_gate, w_up, w_down, top_k):
    assert int(top_k) == 2
    xf = np.ascontiguousarray(x.reshape(N, D).astype(np.float32))
    xf16 = xf.astype(np.float16)

    # ---- phase A: on-device fp16 logits; host top-2 + exact tie repair ----
    global LAST_HW_NS, LAST_PHASE_NS
    LAST_PHASE_NS = {}
    rnc = _get_program("router")
    rwt16 = router_w.T.astype(np.float16)                      # [D, E]
    rwt_r = np.ascontiguousarray(
        rwt16.reshape(KO_D, 128, E).transpose(1, 0, 2))
    in_maps = [
        {"xt": np.ascontiguousarray(
            xf16[c * TSLICE:(c + 1) * TSLICE].T
            .reshape(KO_D, 128, TSLICE).transpose(1, 0, 2)),
         "rwt": rwt_r}
        for c in range(NCORES)
    ]
    rres = _run(rnc, in_maps, "router")
    L = np.concatenate([r["lg"].T for r in rres.results],
                       axis=0).astype(np.float32)              # [N, E]
    order = np.argsort(-L, axis=1, kind="stable")
    l2 = np.take_along_axis(L, order[:, 1:2], 1)[:, 0]
    l3 = np.take_along_axis(L, order[:, 2:3], 1)[:, 0]
    # fp16-logit error is ~4e-4; repair any token whose expert SET could
    # differ from the fp32 reference's top-2 with exact logits.
    bad = np.nonzero(l2 - l3 < 4e-3)[0]
    if bad.size:
        Lx = xf[bad] @ router_w.astype(np.float32).T
        L[bad] = Lx
        order[bad] = np.argsort(-Lx, axis=1, kind="stable")
    e1 = order[:, 0]
    e2 = order[:, 1]
    ar = np.arange(N)
    w1 = (1.0 / (1.0 + np.exp(-(L[ar, e1] - L[ar, e2])))).astype(np.float32)
    w2 = np.float32(1.0) - w1

    # ---- host all-to-all: token rows -> expert cores ----
    toks, wts = [], []
    for e in range(E):
        sel = np.nonzero((e1 == e) | (e2 == e))[0]
        toks.append(sel)
        wts.append(np.where(e1[sel] == e, w1[sel], w2[sel]).astype(np.float32))
    counts = [len(s) for s in toks]
    # every core runs `cap` rows (exec time = slowest core), so use the
    # exact max count instead of rounding up to a multiple of 128
    cap = max(max(counts), 512)

    fnc = _get_program(cap)
    nt = len(_token_tiles(cap))
    in_maps = []
    for e in range(E):
        xgp = np.zeros((nt * 512, D), dtype=np.float16)
        xgp[:counts[e]] = xf16[toks[e]]
        # tile-major [nt, 128, KO_D, 512] so each tile is one contiguous DMA
        xg_t = np.ascontiguousarray(
            xgp.T.reshape(KO_D, 128, nt, 512).transpose(2, 1, 0, 3))
        wgq = _ternary16(np.asarray(w_gate[e], dtype=np.float32))  # [H, D]
        wuq = _ternary16(np.asarray(w_up[e], dtype=np.float32))    # [H, D]
        wdq = _ternary16(np.asarray(w_down[e], dtype=np.float32))  # [D, H]
        wdq *= np.float16(0.25)   # exact; keeps fp16 outputs in range
        in_maps.append({
            "wg": _lhsT_layout(wgq.T, KO_D, HC),
            "wu": _lhsT_layout(wuq.T, KO_D, HC),
            "wd": _lhsT_layout(wdq.T, KO_H, DC),
            "xg": xg_t,
        })
    fres = _run(fnc, in_maps, "ffn")
    if LAST_PHASE_NS:
        LAST_HW_NS = sum(LAST_PHASE_NS.values())

    # ---- unshard: weighted sum of the (<= 2) expert contributions ----
    out = np.zeros((N, D), dtype=np.float32)
    for e in range(E):
        yt_t = fres.results[e]["yt"]                  # [nt, 128, DC, 512]
        y = yt_t.transpose(2, 1, 0, 3).reshape(D, nt * 512)
        ytc = y[:, :counts[e]].T.astype(np.float32)
        out[toks[e]] += (4.0 * wts[e])[:, None] * ytc
    return out.reshape(B, T, D)
